# revision 12
# baseline (speedup 1.0000x reference)
"""CorrelationLayer1D Trainium2 Bass kernel (v3: hybrid band extraction).

Computes out[b, d, h, w] = sum_c x_1[b,c,h,w] * x2p[b,c,h,w+d] for d in [0, 41),
where x2p is x_2 width-padded by (8, 32).  Inputs [4,128,160,320] f32.

Sharding: data-parallel over H = 160 = 8*20 (correlation runs along W only, so
H-sharding needs no halo).  Per core, rows are processed in chunks of HC=10.

Per (b, h) row the W=320 axis splits into blocks M = 128/128/64 with x2 windows
168/168/104.  bf16 Gram matmuls land in PSUM.

Band extraction S[i, (d,h)] = G[i, h, i+d] (note d-major!) is hybrid:
 - blocks 0/1: gpsimd.local_scatter with per-partition indices
   (position (h,j) -> (j-i)*HC + h, out-of-band -> -1 = dropped), on-chip.
 - block 2: atlas stored (j,h)-major, rect DMA to DRAM scratch, skewed reload
   with per-partition pitch+HC step => one contiguous 820B run per partition.
PE transposes per-3-row groups S[:, (d, h-slice)] -> T[(d,h'), w-block] (bf16
PSUM, strided lhsT), one Act/DVE copy casts T into an fp32 SBUF tile [123,320],
and one strided DMA per group writes out[b, :, h:h+3, :] with h innermost so
descriptors spread across DMA engines.
"""

import sys

import numpy as np

try:
    import concourse.bass as bass  # noqa: F401
except ImportError:
    sys.path.insert(0, "/opt/trn_rl_repo")

import concourse.bass as bass
import concourse.tile as tile
from concourse import bacc, masks, mybir
from concourse.ap import AP
from concourse.bass_utils import run_bass_kernel_spmd

MAX_DISP = 40
D = MAX_DISP + 1  # 41 displacements
PAD_L = 8
PAD_R = 32
B, C, H, W = 4, 128, 160, 320
N_CORES = 8
HS = H // N_CORES  # 20 h-rows per core
WP = W + PAD_L + PAD_R  # 360
# (w0, M, window) per block; window = M + MAX_DISP
WBLOCKS = [(0, 128, 168), (128, 128, 168), (256, 64, 104)]

F32 = mybir.dt.float32
BF16 = mybir.dt.bfloat16
I16 = mybir.dt.int16


def build_kernel(b_dim=B, hs=HS, hc=10):
    assert hs % hc == 0
    nchunks = hs // hc

    nc = bacc.Bacc("TRN2", target_bir_lowering=False, debug=False)
    x1e = nc.declare_dram_parameter("x1", [b_dim, C, hs, W], F32, isOutput=False)
    x2e = nc.declare_dram_parameter("x2", [b_dim, C, hs, W], F32, isOutput=False)
    oute = nc.declare_dram_parameter("out", [b_dim, D, hs, W], F32, isOutput=True)

    with tile.TileContext(nc) as tc:
        with (
            tc.tile_pool(name="const", bufs=1) as const_pool,
            tc.tile_pool(name="xf", bufs=2) as xf_pool,
            tc.tile_pool(name="xbf", bufs=2) as xbf_pool,
            tc.tile_pool(name="atl", bufs=3) as atl_pool,
            tc.tile_pool(name="sband", bufs=3) as s_pool,
            tc.tile_pool(name="abg", bufs=2) as abg_pool,
            tc.tile_pool(name="psum_g", bufs=4, space="PSUM") as psum_g,
            tc.tile_pool(name="psum_g2", bufs=2, space="PSUM") as psum_g2,
            tc.tile_pool(name="psum_t", bufs=2, space="PSUM") as psum_t,
            tc.tile_pool(name="scr", bufs=3, space="DRAM") as scr_pool,
        ):
            identity = const_pool.tile([128, 128], BF16)
            masks.make_identity(nc, identity[:])

            # Scatter indices for the 128-wide blocks (window 168), d-major:
            # data position (h, j) in partition i maps to (j-i)*hc + h when
            # 0 <= j-i <= MAX_DISP, else -1 (dropped by local_scatter).
            win128 = 168
            idx128 = const_pool.tile([128, hc * win128], I16, name="idx128")
            nc.gpsimd.iota(
                idx128[:],
                pattern=[[1, hc], [hc, win128]],
                base=0,
                channel_multiplier=-hc,
            )
            nc.gpsimd.affine_select(  # keep where j - i >= 0
                out=idx128[:],
                in_=idx128[:],
                pattern=[[0, hc], [1, win128]],
                compare_op=mybir.AluOpType.is_ge,
                fill=-1,
                base=0,
                channel_multiplier=-1,
            )
            nc.gpsimd.affine_select(  # keep where MAX_DISP - (j - i) >= 0
                out=idx128[:],
                in_=idx128[:],
                pattern=[[0, hc], [-1, win128]],
                compare_op=mybir.AluOpType.is_ge,
                fill=-1,
                base=MAX_DISP,
                channel_multiplier=1,
            )

            for b in range(b_dim):
                for ci in range(nchunks):
                    h0 = ci * hc

                    # ---- load fp32 inputs (contiguous 12.8KB runs) ----
                    x1f = xf_pool.tile([C, hc * W], F32, tag="x1f")
                    nc.sync.dma_start(
                        x1f[:].rearrange("p (h w) -> p h w", w=W),
                        x1e[b, :, h0 : h0 + hc, :],
                    )
                    x2f = xf_pool.tile([C, hc * W], F32, tag="x2f")
                    nc.scalar.dma_start(
                        x2f[:].rearrange("p (h w) -> p h w", w=W),
                        x2e[b, :, h0 : h0 + hc, :],
                    )

                    # ---- convert to bf16 (x2 into padded layout) ----
                    x1b = xbf_pool.tile([C, hc * W], BF16, tag="x1b")
                    nc.scalar.copy(x1b[:], x1f[:])
                    x2b = xbf_pool.tile([C, hc * WP], BF16, tag="x2b")
                    x2b3 = x2b[:].rearrange("p (h w) -> p h w", w=WP)
                    x2f3 = x2f[:].rearrange("p (h w) -> p h w", w=W)
                    nc.gpsimd.memset(x2b3[:, :, 0:PAD_L], 0.0)
                    nc.gpsimd.memset(x2b3[:, :, PAD_L + W : WP], 0.0)
                    half = hc // 2
                    nc.scalar.copy(
                        x2b3[:, 0:half, PAD_L : PAD_L + W], x2f3[:, 0:half, :]
                    )
                    nc.vector.tensor_copy(
                        x2b3[:, half:hc, PAD_L : PAD_L + W], x2f3[:, half:hc, :]
                    )

                    # ---- Gram matmuls -> PSUM -> bf16 atlases ----
                    # atlases 0/1: (h, j)-major; atlas 2: (j, h)-major
                    atl0 = atl_pool.tile([128, hc * 168], BF16, tag="a0")
                    atl1 = atl_pool.tile([128, hc * 168], BF16, tag="a1")
                    atl2 = atl_pool.tile([64, hc * 104], BF16, tag="a2")
                    for h in range(0, hc, 2):
                        for k in (0, 1):
                            w0, _, win = WBLOCKS[k]
                            ps = psum_g.tile([128, 2 * 168], F32, tag="g01")
                            for r in (0, 1):
                                nc.tensor.matmul(
                                    ps[:, r * 168 : (r + 1) * 168],
                                    x1b[:, (h + r) * W + w0 : (h + r) * W + w0 + 128],
                                    x2b[:, (h + r) * WP + w0 : (h + r) * WP + w0 + win],
                                    start=True,
                                    stop=True,
                                )
                            dst = (atl0 if k == 0 else atl1)[
                                :, h * 168 : (h + 2) * 168
                            ]
                            if k == 0:
                                nc.scalar.copy(dst, ps[:])
                            else:
                                nc.vector.tensor_copy(dst, ps[:])
                    w0, _, win = WBLOCKS[2]
                    for h in range(0, hc, 4):
                        rr = min(4, hc - h)
                        ps = psum_g2.tile([64, 4 * 104], F32, tag="g2")
                        for r in range(rr):
                            nc.tensor.matmul(
                                ps[:, r * 104 : (r + 1) * 104],
                                x1b[:, (h + r) * W + w0 : (h + r) * W + w0 + 64],
                                x2b[:, (h + r) * WP + w0 : (h + r) * WP + w0 + win],
                                start=True,
                                stop=True,
                            )
                        # (j, h)-major strided store: element (r, j) -> j*hc + h+r
                        a2dst = atl2[:].rearrange("p (j h) -> p h j", h=hc)[
                            :, h : h + rr, :
                        ]
                        a2src = ps[:, 0 : rr * 104].rearrange(
                            "p (r j) -> p r j", j=104
                        )
                        nc.vector.tensor_copy(a2dst, a2src)

                    # ---- band extraction ----
                    sb0 = s_pool.tile([128, D * hc], BF16, tag="s0")
                    sb1 = s_pool.tile([128, D * hc], BF16, tag="s1")
                    nc.gpsimd.local_scatter(
                        sb0[:],
                        atl0[:],
                        idx128[:],
                        channels=128,
                        num_elems=D * hc,
                        num_idxs=hc * win128,
                    )
                    nc.gpsimd.local_scatter(
                        sb1[:],
                        atl1[:],
                        idx128[:],
                        channels=128,
                        num_elems=D * hc,
                        num_idxs=hc * win128,
                    )
                    # block 2 via DRAM scratch: rect store, skewed contiguous
                    # reload (per-partition pitch hc*104 + hc, 410-elem runs)
                    scr2 = scr_pool.tile([64, hc * 104], BF16, tag="scr2")
                    nc.sync.dma_start(scr2[:], atl2[:])
                    sb2 = s_pool.tile([64, D * hc], BF16, tag="s2")
                    scr_ap = scr2[:]
                    diag = AP(
                        tensor=scr_ap.tensor,
                        offset=scr_ap.offset,
                        ap=[[hc * 104 + hc, 64], [1, D * hc]],
                    )
                    nc.sync.dma_start(sb2[:], diag)
                    sbs = [sb0, sb1, sb2]

                    # ---- PE transpose per row + fp32 out (per-chunk batch) ----
                    abatch = abg_pool.tile([D, hc * W], F32, tag="abatch")
                    for h in range(hc):
                        pst = psum_t.tile([D, W], BF16, tag="t")
                        for k, (w0k, mk, wink) in enumerate(WBLOCKS):
                            lhs = sbs[k][0:mk, :].rearrange(
                                "p (d h) -> p d h", h=hc
                            )[:, :, h : h + 1]
                            nc.tensor.matmul(
                                pst[:, w0k : w0k + mk],
                                lhs,
                                identity[0:mk, 0:mk],
                                start=True,
                                stop=True,
                                is_transpose=True,
                            )
                        if h % 2 == 0:
                            nc.scalar.copy(
                                abatch[:, h * W : (h + 1) * W], pst[:]
                            )
                        else:
                            nc.vector.tensor_copy(
                                abatch[:, h * W : (h + 1) * W], pst[:]
                            )

                    # out[b, d, h0+h, w]: iterate d, h, w => 1280B runs whose
                    # addresses step 5*256B pages in h, spreading DMA engines
                    nc.sync.dma_start(
                        oute[b, :, h0 : h0 + hc, :],
                        abatch[:].rearrange("d (h w) -> d h w", w=W),
                    )

    nc.finalize()
    return nc


_compiled = {}


def _get_kernel(b_dim, hs):
    key = (b_dim, hs)
    if key not in _compiled:
        _compiled[key] = build_kernel(b_dim, hs)
    return _compiled[key]


def kernel(x_1: np.ndarray, x_2: np.ndarray) -> np.ndarray:
    assert x_1.shape == (B, C, H, W) and x_2.shape == (B, C, H, W)
    x_1 = np.ascontiguousarray(x_1, dtype=np.float32)
    x_2 = np.ascontiguousarray(x_2, dtype=np.float32)
    nc = _get_kernel(B, HS)
    in_maps = [
        {
            "x1": np.ascontiguousarray(x_1[:, :, i * HS : (i + 1) * HS, :]),
            "x2": np.ascontiguousarray(x_2[:, :, i * HS : (i + 1) * HS, :]),
        }
        for i in range(N_CORES)
    ]
    res = run_bass_kernel_spmd(nc, in_maps, core_ids=list(range(N_CORES))).results
    out = np.concatenate([res[i]["out"] for i in range(N_CORES)], axis=2)
    return out


# revision 13
# speedup vs baseline: 1.2195x; 1.2195x over previous
"""CorrelationLayer1D Trainium2 Bass kernel (v3: hybrid band extraction).

Computes out[b, d, h, w] = sum_c x_1[b,c,h,w] * x2p[b,c,h,w+d] for d in [0, 41),
where x2p is x_2 width-padded by (8, 32).  Inputs [4,128,160,320] f32.

Sharding: data-parallel over H = 160 = 8*20 (correlation runs along W only, so
H-sharding needs no halo).  Per core, rows are processed in chunks of HC=10.

Per (b, h) row the W=320 axis splits into blocks M = 128/128/64 with x2 windows
168/168/104.  bf16 Gram matmuls land in PSUM.

Band extraction S[i, (d,h)] = G[i, h, i+d] (note d-major!) is hybrid:
 - blocks 0/1: gpsimd.local_scatter with per-partition indices
   (position (h,j) -> (j-i)*HC + h, out-of-band -> -1 = dropped), on-chip.
 - block 2: atlas stored (j,h)-major, rect DMA to DRAM scratch, skewed reload
   with per-partition pitch+HC step => one contiguous 820B run per partition.
PE transposes per-3-row groups S[:, (d, h-slice)] -> T[(d,h'), w-block] (bf16
PSUM, strided lhsT), one Act/DVE copy casts T into an fp32 SBUF tile [123,320],
and one strided DMA per group writes out[b, :, h:h+3, :] with h innermost so
descriptors spread across DMA engines.
"""

import sys

import numpy as np

try:
    import concourse.bass as bass  # noqa: F401
except ImportError:
    sys.path.insert(0, "/opt/trn_rl_repo")

import concourse.bass as bass
import concourse.tile as tile
from concourse import bacc, masks, mybir
from concourse.ap import AP
from concourse.bass_utils import run_bass_kernel_spmd

MAX_DISP = 40
D = MAX_DISP + 1  # 41 displacements
PAD_L = 8
PAD_R = 32
B, C, H, W = 4, 128, 160, 320
N_CORES = 8
HS = H // N_CORES  # 20 h-rows per core
WP = W + PAD_L + PAD_R  # 360
# (w0, M, window) per block; window = M + MAX_DISP
WBLOCKS = [(0, 128, 168), (128, 128, 168), (256, 64, 104)]

F32 = mybir.dt.float32
BF16 = mybir.dt.bfloat16
I16 = mybir.dt.int16


def build_kernel(b_dim=B, hs=HS, hc=10):
    assert hs % hc == 0
    nchunks = hs // hc

    nc = bacc.Bacc("TRN2", target_bir_lowering=False, debug=False)
    x1e = nc.declare_dram_parameter("x1", [b_dim, C, hs, W], F32, isOutput=False)
    x2e = nc.declare_dram_parameter("x2", [b_dim, C, hs, W], F32, isOutput=False)
    oute = nc.declare_dram_parameter("out", [b_dim, D, hs, W], F32, isOutput=True)

    with tile.TileContext(nc) as tc:
        with (
            tc.tile_pool(name="const", bufs=1) as const_pool,
            tc.tile_pool(name="xf", bufs=2) as xf_pool,
            tc.tile_pool(name="xbf", bufs=2) as xbf_pool,
            tc.tile_pool(name="atl", bufs=3) as atl_pool,
            tc.tile_pool(name="sband", bufs=3) as s_pool,
            tc.tile_pool(name="abg", bufs=2) as abg_pool,
            tc.tile_pool(name="psum_g", bufs=4, space="PSUM") as psum_g,
            tc.tile_pool(name="psum_g2", bufs=2, space="PSUM") as psum_g2,
            tc.tile_pool(name="psum_t", bufs=2, space="PSUM") as psum_t,
            tc.tile_pool(name="scr", bufs=3, space="DRAM") as scr_pool,
        ):
            identity = const_pool.tile([128, 128], BF16)
            masks.make_identity(nc, identity[:])

            # Scatter indices, h-major: data position (h, j) in partition i
            # maps to h*D + (j-i) when 0 <= j-i <= MAX_DISP, else -1 (dropped
            # by local_scatter).
            idx_tiles = {}
            for mth, win in ((128, 168), (64, 104)):
                idx = const_pool.tile([mth, hc * win], I16, name=f"idx_{mth}")
                nc.gpsimd.iota(
                    idx[:],
                    pattern=[[D, hc], [1, win]],
                    base=0,
                    channel_multiplier=-1,
                )
                nc.gpsimd.affine_select(  # keep where j - i >= 0
                    out=idx[:],
                    in_=idx[:],
                    pattern=[[0, hc], [1, win]],
                    compare_op=mybir.AluOpType.is_ge,
                    fill=-1,
                    base=0,
                    channel_multiplier=-1,
                )
                nc.gpsimd.affine_select(  # keep where MAX_DISP - (j - i) >= 0
                    out=idx[:],
                    in_=idx[:],
                    pattern=[[0, hc], [-1, win]],
                    compare_op=mybir.AluOpType.is_ge,
                    fill=-1,
                    base=MAX_DISP,
                    channel_multiplier=1,
                )
                idx_tiles[mth] = idx

            for b in range(b_dim):
                for ci in range(nchunks):
                    h0 = ci * hc

                    # ---- load fp32 inputs (contiguous 12.8KB runs) ----
                    x1f = xf_pool.tile([C, hc * W], F32, tag="x1f")
                    nc.sync.dma_start(
                        x1f[:].rearrange("p (h w) -> p h w", w=W),
                        x1e[b, :, h0 : h0 + hc, :],
                    )
                    x2f = xf_pool.tile([C, hc * W], F32, tag="x2f")
                    nc.scalar.dma_start(
                        x2f[:].rearrange("p (h w) -> p h w", w=W),
                        x2e[b, :, h0 : h0 + hc, :],
                    )

                    # ---- convert to bf16 (x2 into padded layout) ----
                    x1b = xbf_pool.tile([C, hc * W], BF16, tag="x1b")
                    nc.scalar.copy(x1b[:], x1f[:])
                    x2b = xbf_pool.tile([C, hc * WP], BF16, tag="x2b")
                    x2b3 = x2b[:].rearrange("p (h w) -> p h w", w=WP)
                    x2f3 = x2f[:].rearrange("p (h w) -> p h w", w=W)
                    nc.gpsimd.memset(x2b3[:, :, 0:PAD_L], 0.0)
                    nc.gpsimd.memset(x2b3[:, :, PAD_L + W : WP], 0.0)
                    half = hc // 2
                    nc.scalar.copy(
                        x2b3[:, 0:half, PAD_L : PAD_L + W], x2f3[:, 0:half, :]
                    )
                    nc.vector.tensor_copy(
                        x2b3[:, half:hc, PAD_L : PAD_L + W], x2f3[:, half:hc, :]
                    )

                    # ---- Gram matmuls -> PSUM -> bf16 atlases ----
                    # atlases 0/1: (h, j)-major; atlas 2: (j, h)-major
                    atl0 = atl_pool.tile([128, hc * 168], BF16, tag="a0")
                    atl1 = atl_pool.tile([128, hc * 168], BF16, tag="a1")
                    atl2 = atl_pool.tile([64, hc * 104], BF16, tag="a2")
                    for h in range(0, hc, 2):
                        for k in (0, 1):
                            w0, _, win = WBLOCKS[k]
                            ps = psum_g.tile([128, 2 * 168], F32, tag="g01")
                            for r in (0, 1):
                                nc.tensor.matmul(
                                    ps[:, r * 168 : (r + 1) * 168],
                                    x1b[:, (h + r) * W + w0 : (h + r) * W + w0 + 128],
                                    x2b[:, (h + r) * WP + w0 : (h + r) * WP + w0 + win],
                                    start=True,
                                    stop=True,
                                )
                            dst = (atl0 if k == 0 else atl1)[
                                :, h * 168 : (h + 2) * 168
                            ]
                            if k == 0:
                                nc.scalar.copy(dst, ps[:])
                            else:
                                nc.vector.tensor_copy(dst, ps[:])
                    w0, _, win = WBLOCKS[2]
                    for h in range(0, hc, 4):
                        rr = min(4, hc - h)
                        ps = psum_g2.tile([64, 4 * 104], F32, tag="g2")
                        for r in range(rr):
                            nc.tensor.matmul(
                                ps[:, r * 104 : (r + 1) * 104],
                                x1b[:, (h + r) * W + w0 : (h + r) * W + w0 + 64],
                                x2b[:, (h + r) * WP + w0 : (h + r) * WP + w0 + win],
                                start=True,
                                stop=True,
                            )
                        nc.vector.tensor_copy(
                            atl2[:, h * 104 : (h + rr) * 104], ps[:, 0 : rr * 104]
                        )

                    # ---- band extraction: on-chip per-partition scatter ----
                    sb0 = s_pool.tile([128, hc * D], BF16, tag="s0")
                    sb1 = s_pool.tile([128, hc * D], BF16, tag="s1")
                    sb2 = s_pool.tile([64, hc * D], BF16, tag="s2")
                    sbs = [sb0, sb1, sb2]
                    for k, (w0k, mk, wink) in enumerate(WBLOCKS):
                        nc.gpsimd.local_scatter(
                            sbs[k][:],
                            (atl0, atl1, atl2)[k][:],
                            idx_tiles[mk][:],
                            channels=mk,
                            num_elems=hc * D,
                            num_idxs=hc * wink,
                        )

                    # ---- PE transpose per row + fp32 out (per-chunk batch) ----
                    abatch = abg_pool.tile([D, hc * W], F32, tag="abatch")
                    for h in range(hc):
                        pst = psum_t.tile([D, W], BF16, tag="t")
                        for k, (w0k, mk, wink) in enumerate(WBLOCKS):
                            lhs = sbs[k][0:mk, h * D : (h + 1) * D]
                            nc.tensor.matmul(
                                pst[:, w0k : w0k + mk],
                                lhs,
                                identity[0:mk, 0:mk],
                                start=True,
                                stop=True,
                                is_transpose=True,
                            )
                        if h % 2 == 0:
                            nc.scalar.copy(
                                abatch[:, h * W : (h + 1) * W], pst[:]
                            )
                        else:
                            nc.vector.tensor_copy(
                                abatch[:, h * W : (h + 1) * W], pst[:]
                            )

                    # out[b, d, h0+h, w]: iterate d, h, w => 1280B runs whose
                    # addresses step 5*256B pages in h, spreading DMA engines
                    nc.sync.dma_start(
                        oute[b, :, h0 : h0 + hc, :],
                        abatch[:].rearrange("d (h w) -> d h w", w=W),
                    )

    nc.finalize()
    return nc


_compiled = {}


def _get_kernel(b_dim, hs):
    key = (b_dim, hs)
    if key not in _compiled:
        _compiled[key] = build_kernel(b_dim, hs)
    return _compiled[key]


def kernel(x_1: np.ndarray, x_2: np.ndarray) -> np.ndarray:
    assert x_1.shape == (B, C, H, W) and x_2.shape == (B, C, H, W)
    x_1 = np.ascontiguousarray(x_1, dtype=np.float32)
    x_2 = np.ascontiguousarray(x_2, dtype=np.float32)
    nc = _get_kernel(B, HS)
    in_maps = [
        {
            "x1": np.ascontiguousarray(x_1[:, :, i * HS : (i + 1) * HS, :]),
            "x2": np.ascontiguousarray(x_2[:, :, i * HS : (i + 1) * HS, :]),
        }
        for i in range(N_CORES)
    ]
    res = run_bass_kernel_spmd(nc, in_maps, core_ids=list(range(N_CORES))).results
    out = np.concatenate([res[i]["out"] for i in range(N_CORES)], axis=2)
    return out


# revision 14
# speedup vs baseline: 1.3572x; 1.1129x over previous
"""CorrelationLayer1D Trainium2 Bass kernel (v3: hybrid band extraction).

Computes out[b, d, h, w] = sum_c x_1[b,c,h,w] * x2p[b,c,h,w+d] for d in [0, 41),
where x2p is x_2 width-padded by (8, 32).  Inputs [4,128,160,320] f32.

Sharding: data-parallel over H = 160 = 8*20 (correlation runs along W only, so
H-sharding needs no halo).  Per core, rows are processed in chunks of HC=10.

Per (b, h) row the W=320 axis splits into blocks M = 128/128/64 with x2 windows
168/168/104.  bf16 Gram matmuls land in PSUM.

Band extraction S[i, (d,h)] = G[i, h, i+d] (note d-major!) is hybrid:
 - blocks 0/1: gpsimd.local_scatter with per-partition indices
   (position (h,j) -> (j-i)*HC + h, out-of-band -> -1 = dropped), on-chip.
 - block 2: atlas stored (j,h)-major, rect DMA to DRAM scratch, skewed reload
   with per-partition pitch+HC step => one contiguous 820B run per partition.
PE transposes per-3-row groups S[:, (d, h-slice)] -> T[(d,h'), w-block] (bf16
PSUM, strided lhsT), one Act/DVE copy casts T into an fp32 SBUF tile [123,320],
and one strided DMA per group writes out[b, :, h:h+3, :] with h innermost so
descriptors spread across DMA engines.
"""

import sys

import numpy as np

try:
    import concourse.bass as bass  # noqa: F401
except ImportError:
    sys.path.insert(0, "/opt/trn_rl_repo")

import concourse.bass as bass
import concourse.tile as tile
from concourse import bacc, masks, mybir
from concourse.ap import AP
from concourse.bass_utils import run_bass_kernel_spmd

MAX_DISP = 40
D = MAX_DISP + 1  # 41 displacements
PAD_L = 8
PAD_R = 32
B, C, H, W = 4, 128, 160, 320
N_CORES = 8
HS = H // N_CORES  # 20 h-rows per core
WP = W + PAD_L + PAD_R  # 360
# (w0, M, window) per block; window = M + MAX_DISP
WBLOCKS = [(0, 128, 168), (128, 128, 168), (256, 64, 104)]

F32 = mybir.dt.float32
BF16 = mybir.dt.bfloat16
I16 = mybir.dt.int16


def build_kernel(b_dim=B, hs=HS, hc=10):
    assert hs % hc == 0
    nchunks = hs // hc

    nc = bacc.Bacc("TRN2", target_bir_lowering=False, debug=False)
    x1e = nc.declare_dram_parameter("x1", [b_dim, C, hs, W], F32, isOutput=False)
    x2e = nc.declare_dram_parameter("x2", [b_dim, C, hs, W], F32, isOutput=False)
    oute = nc.declare_dram_parameter("out", [b_dim, D, hs, W], F32, isOutput=True)

    with tile.TileContext(nc) as tc:
        with (
            tc.tile_pool(name="const", bufs=1) as const_pool,
            tc.tile_pool(name="xf", bufs=2) as xf_pool,
            tc.tile_pool(name="xbf", bufs=2) as xbf_pool,
            tc.tile_pool(name="atl", bufs=3) as atl_pool,
            tc.tile_pool(name="sband", bufs=3) as s_pool,
            tc.tile_pool(name="abg", bufs=2) as abg_pool,
            tc.tile_pool(name="psum_g", bufs=4, space="PSUM") as psum_g,
            tc.tile_pool(name="psum_g2", bufs=2, space="PSUM") as psum_g2,
            tc.tile_pool(name="psum_t", bufs=2, space="PSUM") as psum_t,
            tc.tile_pool(name="scr", bufs=3, space="DRAM") as scr_pool,
        ):
            identity = const_pool.tile([128, 128], BF16)
            masks.make_identity(nc, identity[:])

            # Scatter indices, h-major: data position (h, j) in partition i
            # maps to h*D + (j-i) when 0 <= j-i <= MAX_DISP, else -1 (dropped
            # by local_scatter).
            idx_tiles = {}
            for mth, win in ((128, 168), (64, 104)):
                idx = const_pool.tile([mth, hc * win], I16, name=f"idx_{mth}")
                nc.gpsimd.iota(
                    idx[:],
                    pattern=[[D, hc], [1, win]],
                    base=0,
                    channel_multiplier=-1,
                )
                nc.gpsimd.affine_select(  # keep where j - i >= 0
                    out=idx[:],
                    in_=idx[:],
                    pattern=[[0, hc], [1, win]],
                    compare_op=mybir.AluOpType.is_ge,
                    fill=-1,
                    base=0,
                    channel_multiplier=-1,
                )
                nc.gpsimd.affine_select(  # keep where MAX_DISP - (j - i) >= 0
                    out=idx[:],
                    in_=idx[:],
                    pattern=[[0, hc], [-1, win]],
                    compare_op=mybir.AluOpType.is_ge,
                    fill=-1,
                    base=MAX_DISP,
                    channel_multiplier=1,
                )
                idx_tiles[mth] = idx

            for b in range(b_dim):
                for ci in range(nchunks):
                    h0 = ci * hc

                    # ---- load fp32 inputs (contiguous 12.8KB runs) ----
                    x1f = xf_pool.tile([C, hc * W], F32, tag="x1f")
                    nc.sync.dma_start(
                        x1f[:].rearrange("p (h w) -> p h w", w=W),
                        x1e[b, :, h0 : h0 + hc, :],
                    )
                    x2f = xf_pool.tile([C, hc * W], F32, tag="x2f")
                    nc.scalar.dma_start(
                        x2f[:].rearrange("p (h w) -> p h w", w=W),
                        x2e[b, :, h0 : h0 + hc, :],
                    )

                    # ---- convert to bf16 (x2 into padded layout) ----
                    x1b = xbf_pool.tile([C, hc * W], BF16, tag="x1b")
                    nc.scalar.copy(x1b[:], x1f[:])
                    x2b = xbf_pool.tile([C, hc * WP], BF16, tag="x2b")
                    x2b3 = x2b[:].rearrange("p (h w) -> p h w", w=WP)
                    x2f3 = x2f[:].rearrange("p (h w) -> p h w", w=W)
                    nc.gpsimd.memset(x2b3[:, :, 0:PAD_L], 0.0)
                    nc.gpsimd.memset(x2b3[:, :, PAD_L + W : WP], 0.0)
                    half = hc // 2
                    nc.scalar.copy(
                        x2b3[:, 0:half, PAD_L : PAD_L + W], x2f3[:, 0:half, :]
                    )
                    nc.vector.tensor_copy(
                        x2b3[:, half:hc, PAD_L : PAD_L + W], x2f3[:, half:hc, :]
                    )

                    # ---- Gram matmuls -> PSUM -> bf16 atlases ----
                    # atlases 0/1: (h, j)-major; atlas 2: (j, h)-major
                    atl0 = atl_pool.tile([128, hc * 168], BF16, tag="a0")
                    atl1 = atl_pool.tile([128, hc * 168], BF16, tag="a1")
                    atl2 = atl_pool.tile([64, hc * 104], BF16, tag="a2")
                    for h in range(0, hc, 2):
                        for k in (0, 1):
                            w0, _, win = WBLOCKS[k]
                            ps = psum_g.tile([128, 2 * 168], F32, tag="g01")
                            for r in (0, 1):
                                nc.tensor.matmul(
                                    ps[:, r * 168 : (r + 1) * 168],
                                    x1b[:, (h + r) * W + w0 : (h + r) * W + w0 + 128],
                                    x2b[:, (h + r) * WP + w0 : (h + r) * WP + w0 + win],
                                    start=True,
                                    stop=True,
                                )
                            dst = (atl0 if k == 0 else atl1)[
                                :, h * 168 : (h + 2) * 168
                            ]
                            if k == 0:
                                nc.scalar.copy(dst, ps[:])
                            else:
                                nc.vector.tensor_copy(dst, ps[:])
                    w0, _, win = WBLOCKS[2]
                    for h in range(0, hc, 4):
                        rr = min(4, hc - h)
                        ps = psum_g2.tile([64, 4 * 104], F32, tag="g2")
                        for r in range(rr):
                            nc.tensor.matmul(
                                ps[:, r * 104 : (r + 1) * 104],
                                x1b[:, (h + r) * W + w0 : (h + r) * W + w0 + 64],
                                x2b[:, (h + r) * WP + w0 : (h + r) * WP + w0 + win],
                                start=True,
                                stop=True,
                            )
                        nc.vector.tensor_copy(
                            atl2[:, h * 104 : (h + rr) * 104], ps[:, 0 : rr * 104]
                        )

                    # ---- band extraction: on-chip per-partition scatter ----
                    sb0 = s_pool.tile([128, hc * D], BF16, tag="s0")
                    sb1 = s_pool.tile([128, hc * D], BF16, tag="s1")
                    sb2 = s_pool.tile([64, hc * D], BF16, tag="s2")
                    sbs = [sb0, sb1, sb2]
                    for k, (w0k, mk, wink) in enumerate(WBLOCKS):
                        nc.gpsimd.local_scatter(
                            sbs[k][:],
                            (atl0, atl1, atl2)[k][:],
                            idx_tiles[mk][:],
                            channels=mk,
                            num_elems=hc * D,
                            num_idxs=hc * wink,
                        )

                    # ---- PE transpose per row + fp32 out (per-chunk batch) ----
                    abatch = abg_pool.tile([D, hc * W], F32, tag="abatch")
                    for h in range(hc):
                        pst = psum_t.tile([D, W], BF16, tag="t")
                        for k, (w0k, mk, wink) in enumerate(WBLOCKS):
                            lhs = sbs[k][0:mk, h * D : (h + 1) * D]
                            nc.tensor.matmul(
                                pst[:, w0k : w0k + mk],
                                lhs,
                                identity[0:mk, 0:mk],
                                start=True,
                                stop=True,
                                is_transpose=True,
                            )
                        if h % 2 == 0:
                            nc.scalar.copy(
                                abatch[:, h * W : (h + 1) * W], pst[:]
                            )
                        else:
                            nc.vector.tensor_copy(
                                abatch[:, h * W : (h + 1) * W], pst[:]
                            )

                    # out[b, d, h0+h, w]: iterate d, h, w => 1280B runs whose
                    # addresses step 5*256B pages in h, spreading DMA engines
                    nc.gpsimd.dma_start(
                        oute[b, :, h0 : h0 + hc, :],
                        abatch[:].rearrange("d (h w) -> d h w", w=W),
                    )

    nc.finalize()
    return nc


_compiled = {}


def _get_kernel(b_dim, hs):
    key = (b_dim, hs)
    if key not in _compiled:
        _compiled[key] = build_kernel(b_dim, hs)
    return _compiled[key]


def kernel(x_1: np.ndarray, x_2: np.ndarray) -> np.ndarray:
    assert x_1.shape == (B, C, H, W) and x_2.shape == (B, C, H, W)
    x_1 = np.ascontiguousarray(x_1, dtype=np.float32)
    x_2 = np.ascontiguousarray(x_2, dtype=np.float32)
    nc = _get_kernel(B, HS)
    in_maps = [
        {
            "x1": np.ascontiguousarray(x_1[:, :, i * HS : (i + 1) * HS, :]),
            "x2": np.ascontiguousarray(x_2[:, :, i * HS : (i + 1) * HS, :]),
        }
        for i in range(N_CORES)
    ]
    res = run_bass_kernel_spmd(nc, in_maps, core_ids=list(range(N_CORES))).results
    out = np.concatenate([res[i]["out"] for i in range(N_CORES)], axis=2)
    return out
